# revision 23
# baseline (speedup 1.0000x reference)
"""Bahdanau-style attention kernel for Trainium2, data-parallel over batch
across 8 NeuronCores.

Reference computation (per batch b):
    W_h, W_e = W_attn[:H], W_attn[H:]
    proj   = hidden @ W_h + enc[b] @ W_e + b_attn          # [S, H]
    energy = tanh(proj)
    scores = energy @ W_v                                   # [S]
    scores = where(mask==0, -1e10, scores)
    attn   = softmax(scores)
    ctx    = attn @ enc[b]                                  # [2H]

Shapes: B=32, S=1024, H=512, 2H=1024.  8 cores x 4 batches each.

Per-core implementation (v1):
  - enc rows are cast to bf16 during the HBM->SBUF DMA (gpsimd SWDGE
    cast-dma), halving SBUF residency and making the PE transposes run
    at 1.0 cycles/row instead of 1.5 (f32r).
  - The e-contraction (enc @ W_e) needs e on partitions, so each 128x128
    bf16 tile is transposed on the TensorEngine and the PSUM result
    copied back to SBUF (copies split across Act and Pool engines).
  - The bias row hb[b] = (hidden @ W_h + b_attn)[b] is PRELOADED into the
    PSUM bank by a Pool broadcast copy; the 8 main matmuls accumulate on
    top of it (start=False), so no PE cycles are spent on bias.
  - scores = reduce(tanh(proj) * wv) is one fused DVE tensor_tensor_reduce.
  - Softmax without running max: masked scores are -1e10 so exp
    underflows to 0; denominators stay finite.
  - context: enc tiles (natural [s,e] layout, bf16) are the STATIONARY
    matmul operand and the unnormalized p column [128,1] streams through:
    64 ap-size-1 matmuls (~4 cyc each) instead of 16 ap-512 matmuls.
    The [128e, 8] PSUM result is transposed once and scaled by 1/denom.
"""

import numpy as np

B, S, H = 32, 1024, 512
E = 2 * H            # 1024
NEG = -1e10
N_CORES = 8
B_LOC = B // N_CORES  # 4
ST = S // 128         # 8 s-tiles per batch
ET = E // 128         # 8 e-tiles (k-tiles of the main matmul)
KT_H = H // 128       # 4 k-tiles for hidden @ W_h

_cache = {}
_USE_TTR = False


def _install_tile_drain_patch():
    """walrus in this container rejects >1 sem-wait on the SP CTRL drain that
    TileContext emits at kernel tail; split the waits across 1-wait nops."""
    import concourse.tile as tile
    import concourse.mybir as mybir
    from concourse.vector_clock import ScopedClock

    if getattr(tile.TileContext, "_drain_patch_installed", False):
        return

    def _drain_and_barrier_split(self, tick_clock, wait_clock):
        nc = self.nc
        probe = nc.sync.nop(nofuse=True, hint="tail_wait_probe")
        wait_clock.add_sem_waits(
            probe.ins, ScopedClock({None: tick_clock.global_clock})
        )
        si = probe.ins.sync_info
        waits = list(si.on_wait) if si and si.on_wait else []
        if len(waits) > 1:
            si.on_wait = waits[:1]
            for w in waits[1:]:
                n = nc.sync.nop(nofuse=True, hint="tail_wait_extra")
                nsi = n.ins.sync_info
                if nsi is None:
                    n.ins.sync_info = mybir.SyncInfo(on_wait=[w], on_update=[])
                else:
                    nsi.on_wait = [w]
        nc.sync.drain()
        nc.all_engine_barrier()
        assert self.sems is not None
        popped = nc._tile_sem_poison_stack.pop()
        assert popped is self._sem_poison
        # chunked clear_and_free_semaphores: walrus rejects RANGE_CLEAR ISA
        # instructions spanning more than a few semaphores ("ISA wrong
        # length"), so clear in <=3-wide ranges.
        sems = list(self.sems.allocated().values())
        sem_nums = sorted(s.num if hasattr(s, "num") else s for s in sems)
        if sem_nums:
            runs = []
            lo = prev = sem_nums[0]
            for n in sem_nums[1:]:
                if n == prev + 1:
                    prev = n
                else:
                    runs.append((lo, prev))
                    lo = prev = n
            runs.append((lo, prev))
            for lo, hi in runs:
                for c0 in range(lo, hi + 1, 3):
                    c1 = min(c0 + 2, hi)
                    r = range(c0, c1 + 1)
                    assert nc._state.free_isdisjoint(r)
                    nc.gpsimd.dma_reset(r)
                    nc.gpsimd.sem_clear(r)
            nc._state.prepend_free_semaphores(sem_nums)
            for poison_set in nc._tile_sem_poison_stack:
                poison_set.update(sem_nums)
        nc.all_engine_barrier()

    tile.TileContext._drain_and_barrier = _drain_and_barrier_split
    tile.TileContext._drain_patch_installed = True


def _split_multiwaits(nc, max_waits=1):
    """walrus's setupSyncWait rejects instructions carrying more than a couple
    of semaphore waits.  Move excess waits onto same-engine nops inserted
    immediately before the offending instruction (engine executes in order, so
    semantics are identical)."""
    import concourse.mybir as mybir

    for f in nc.m.functions:
        for bb in f.blocks:
            out = []
            for inst in bb.instructions:
                si = inst.sync_info
                waits = list(si.on_wait) if si and si.on_wait else []
                lim = max_waits
                if len(waits) > lim:
                    excess = waits[:-lim]
                    si.on_wait = waits[-lim:]
                    for i in range(0, len(excess), max_waits):
                        nop = mybir.InstNoOp(
                            name=f"I-{nc.next_id()}-waitsplit", ins=[], outs=[]
                        )
                        nop.engine = inst.engine
                        nop.sync_info = mybir.SyncInfo(
                            on_wait=excess[i:i + max_waits], on_update=[]
                        )
                        nc.register_instruction(nop, overwrite=True)
                        out.append(nop)
                out.append(inst)
            bb.instructions[:] = out


def _emit_context(nc, mybir, pools, pends, ctx_d):
    """Deferred context emission: ctx(b) = (enc_b.T-stationary @ p_col)/denom.
    64 ap-size-1 matmuls accumulate into one [128, ET] PSUM tile; the tile is
    transposed (PE) and scaled by 1/denom on the way out."""
    f32 = mybir.dt.float32
    bf16 = mybir.dt.bfloat16
    pctx_pool, ctxpool, id16 = pools
    for (b, enc_b, p_r16, rb) in pends:
        p_ctx = pctx_pool.tile([128, ET], f32, tag="p_ctx")
        for j in range(ET):
            for st in range(ST):
                nc.tensor.matmul(
                    p_ctx[:, j:j + 1],
                    enc_b[:, st * E + j * 128: st * E + (j + 1) * 128],
                    p_r16[:, st:st + 1],
                    start=(st == 0), stop=(st == ST - 1),
                )
        ctx_sb = ctxpool.tile([128, ET], bf16, tag="ctx_sb")
        nc.vector.tensor_copy(ctx_sb[:], p_ctx[:])
        p_ctxT = pctx_pool.tile([ET, 128], bf16, tag="p_ctxT")
        nc.tensor.transpose(p_ctxT[:], ctx_sb[:], id16[:])
        ctxT = ctxpool.tile([ET, 128], f32, tag="ctxT")
        nc.vector.tensor_scalar_mul(ctxT[:], p_ctxT[:], rb[0:ET, :])
        nc.sync.dma_start(
            ctx_d[b].rearrange("(j p) -> j p", j=ET), ctxT[:]
        )


def build_kernel(n_iters: int = 1, debug_dumps: bool = False):
    """Build the per-core Bass program.  n_iters>1 repeats the whole compute
    body (for slope-based wall-clock timing); outputs are just rewritten."""
    _install_tile_drain_patch()
    import concourse.bass as bass
    import concourse.tile as tile
    import concourse.mybir as mybir
    from concourse.mybir import AluOpType as alu
    from concourse.mybir import ActivationFunctionType as act

    f32 = mybir.dt.float32
    f32r = mybir.dt.float32r
    bf16 = mybir.dt.bfloat16
    i32 = mybir.dt.int32

    nc = bass.Bass("TRN2", target_bir_lowering=False, debug=False,
                   num_devices=N_CORES)

    hidden_d = nc.dram_tensor("hidden", [B_LOC, H], f32r, kind="ExternalInput").ap()
    enc_d = nc.dram_tensor("enc", [B_LOC, S, E], f32, kind="ExternalInput").ap()
    mask_d = nc.dram_tensor("mask", [B_LOC, S], i32, kind="ExternalInput").ap()
    wattn_d = nc.dram_tensor("w_attn", [3 * H, H], f32r, kind="ExternalInput").ap()
    battn_d = nc.dram_tensor("b_attn", [H], f32r, kind="ExternalInput").ap()
    wv_d = nc.dram_tensor("w_v", [H], f32, kind="ExternalInput").ap()
    ctx_d = nc.dram_tensor("out_ctx", [B_LOC, E], f32, kind="ExternalOutput").ap()
    attn_d = nc.dram_tensor("out_attn", [B_LOC, S], f32, kind="ExternalOutput").ap()
    dbg_d = None
    if debug_dumps:
        dbg_d = nc.dram_tensor("dbg_scores", [B_LOC, S], f32,
                               kind="ExternalOutput").ap()
        dbg_hb = nc.dram_tensor("dbg_hb", [1, B_LOC * H], f32,
                                kind="ExternalOutput").ap()
        dbg_enc = nc.dram_tensor("dbg_enc", [128, E], f32,
                                 kind="ExternalOutput").ap()
        dbg_encT = nc.dram_tensor("dbg_encT", [128, E], f32,
                                  kind="ExternalOutput").ap()
        dbg_proj = nc.dram_tensor("dbg_proj", [128, H], f32,
                                  kind="ExternalOutput").ap()

    with tile.TileContext(nc) as tc:
        with (
            tc.tile_pool(name="const", bufs=1) as cpool,
            tc.tile_pool(name="enc", bufs=3) as encpool,
            tc.tile_pool(name="encT", bufs=7) as encTpool,
            tc.tile_pool(name="work", bufs=4) as wpool,
            tc.tile_pool(name="perb", bufs=3) as bpool,
            tc.tile_pool(name="ctxp", bufs=2) as ctxpool,
            tc.tile_pool(name="ptr", bufs=2, space="PSUM") as ptr_pool,
            tc.tile_pool(name="pproj", bufs=3, space="PSUM") as pproj_pool,
            tc.tile_pool(name="pctx", bufs=1, space="PSUM") as pctx_pool,
            tc.tile_pool(name="psmall", bufs=1, space="PSUM") as psmall_pool,
        ):
            # ---------------- constants / weights ----------------
            ones_f = cpool.tile([128, 128], f32)
            nc.vector.memset(ones_f[:], 1.0)
            ones_col_f = cpool.tile([128, 1], f32)
            nc.vector.memset(ones_col_f[:], 1.0)
            ones_row_f = cpool.tile([1, 128], f32)
            nc.vector.memset(ones_row_f[:], 1.0)

            id_f = cpool.tile([128, 128], f32)
            nc.gpsimd.affine_select(
                id_f[:], ones_f[:], pattern=[[1, 128]],
                compare_op=alu.is_equal, fill=0.0, base=0,
                channel_multiplier=-1,
            )
            id16 = cpool.tile([128, 128], bf16)
            nc.vector.tensor_copy(id16[:], id_f[:])

            # W_e (rows H..3H of W_attn) as bf16: [128, ET*512]
            w_e = cpool.tile([128, ET * H], bf16)
            nc.gpsimd.dma_start(
                w_e[:].rearrange("p (kt h) -> p kt h", h=H),
                wattn_d[H:3 * H, :].rearrange("(kt p) h -> p kt h", p=128),
            )
            # W_h: rows 0..H -> [128, KT_H*512] (f32r, full precision)
            w_h = cpool.tile([128, KT_H * H], f32r)
            nc.sync.dma_start(
                w_h[:].rearrange("p (kt h) -> p kt h", h=H),
                wattn_d[0:H, :].rearrange("(kt p) h -> p kt h", p=128),
            )
            # hidden transposed: [128, KT_H*B_LOC]
            hidT = cpool.tile([128, KT_H * B_LOC], f32r)
            for kt in range(KT_H):
                nc.sync.dma_start(
                    hidT[:, kt * B_LOC:(kt + 1) * B_LOC],
                    hidden_d[:, kt * 128:(kt + 1) * 128].rearrange("b p -> p b"),
                )
            wv_row = cpool.tile([1, H], f32)
            nc.sync.dma_start(wv_row[:], wv_d[None, :])
            battn_row = cpool.tile([1, H], f32r)
            nc.sync.dma_start(battn_row[:], battn_d[None, :])

            # broadcast W_v to all partitions (bf16 for the fused DVE reduce)
            p_wv = psmall_pool.tile([128, H], f32, tag="small")
            nc.tensor.matmul(p_wv[:], ones_row_f[:], wv_row[:],
                             start=True, stop=True)
            wv_bc = cpool.tile([128, H], bf16)
            nc.any.tensor_copy(wv_bc[:], p_wv[:])

            # hb = hidden @ W_h + b_attn  (rows 0..3 of PSUM), kept f32
            ones_tl = cpool.tile([1, B_LOC], f32r)
            nc.vector.tensor_copy(ones_tl[:], ones_f[0:1, 0:B_LOC])
            p_ph = psmall_pool.tile([B_LOC, H], f32, tag="small")
            for kt in range(KT_H):
                nc.tensor.matmul(
                    p_ph[:],
                    hidT[:, kt * B_LOC:(kt + 1) * B_LOC],
                    w_h[:, kt * H:(kt + 1) * H],
                    start=(kt == 0), stop=False,
                )
            nc.tensor.matmul(p_ph[:], ones_tl[:],
                             battn_row[:], start=False, stop=True)
            hb = cpool.tile([B_LOC, H], f32r)
            nc.any.tensor_copy(hb[:], p_ph[:])
            # all four hb rows gathered onto partition 0 (matmul rhs needs
            # base partition 0); SBUF->SBUF DMA, 4 descriptors, setup-only.
            hb_flat = cpool.tile([1, B_LOC * H], f32r)
            nc.sync.dma_start(
                hb_flat[:].rearrange("p (b h) -> p b h", h=H), hb[:]
            )
            ones_row_r = cpool.tile([1, 128], f32r)
            nc.vector.tensor_copy(ones_row_r[:], ones_row_f[:])

            # masks -> additive bias: 0 where mask==1, -1e10 where mask==0
            mbias = cpool.tile([128, B_LOC * ST], f32)
            for b in range(B_LOC):
                mf = wpool.tile([128, ST], f32, tag="maskf")
                nc.gpsimd.dma_start(
                    mf[:], mask_d[b].rearrange("(st p) -> p st", p=128)
                )
                nc.vector.tensor_scalar(
                    mbias[:, b * ST:(b + 1) * ST], mf[:], 1e10, -1e10,
                    op0=alu.mult, op1=alu.add,
                )

            # ---------------- main loop ----------------
            ctx_pools = (pctx_pool, ctxpool, id16)
            for it in range(n_iters):
                pend = []  # [(b, enc_b, p_r16, rb)] awaiting context emission
                for b in range(B_LOC):
                    # enc cast-loaded to bf16 via gpsimd SWDGE dma
                    enc_b = encpool.tile([128, ST * E], bf16, tag="enc_b")
                    for h2 in range(2):
                        nc.gpsimd.dma_start(
                            enc_b[:, h2 * 4 * E:(h2 + 1) * 4 * E].rearrange(
                                "p (st e) -> p st e", e=E),
                            enc_d[b][h2 * 512:(h2 + 1) * 512, :].rearrange(
                                "(st p) e -> p st e", p=128),
                        )

                    s_sb = bpool.tile([128, ST], f32, tag="s_sb")
                    # Two-stage software pipeline over s-tiles: transposes for
                    # tile st+1 are emitted BEFORE the proj matmuls of tile st.
                    encT_q = []
                    for st in range(ST + 1):
                        if st < ST:
                            encT = encTpool.tile([128, E], bf16, tag="encT")
                            for g in range(2):
                                p_tr = ptr_pool.tile([128, 512], bf16, tag="p_tr")
                                for j4 in range(4):
                                    j = g * 4 + j4
                                    nc.tensor.transpose(
                                        p_tr[:, j4 * 128:(j4 + 1) * 128],
                                        enc_b[:, st * E + j * 128: st * E + (j + 1) * 128],
                                        id16[:],
                                    )
                                nc.vector.tensor_copy(
                                    encT[:, g * 512:(g + 1) * 512], p_tr[:]
                                )
                            encT_q.append(encT)
                        if st == 0:
                            continue
                        stp = st - 1
                        encT_p = encT_q.pop(0)
                        # PSUM preload of the bias row (Pool broadcast copy),
                        # then proj matmuls accumulate on top (start=False).
                        p_proj = pproj_pool.tile([128, H], f32, tag="p_proj")
                        # K=1 broadcast matmul seeds the bank with the bias
                        # row (hb[b] broadcast to all 128 partitions).
                        nc.tensor.matmul(
                            p_proj[:], ones_row_r[:],
                            hb_flat[:, b * H:(b + 1) * H],
                            start=True, stop=False,
                        )
                        for j in range(ET):
                            nc.tensor.matmul(
                                p_proj[:],
                                encT_p[:, j * 128:(j + 1) * 128],
                                w_e[:, j * H:(j + 1) * H],
                                start=False, stop=(j == ET - 1),
                            )
                        if debug_dumps and it == 0 and b == 0 and stp == 0:
                            nc.gpsimd.dma_start(dbg_hb[:, :], hb_flat[:])
                            nc.gpsimd.dma_start(dbg_enc[:, :], enc_b[:, 0:E])
                            nc.gpsimd.dma_start(dbg_encT[:, :], encT_p[:])
                            dproj = wpool.tile([128, H], f32, tag="dproj")
                            nc.vector.tensor_copy(dproj[:], p_proj[:])
                            nc.sync.dma_start(dbg_proj[:, :], dproj[:])
                        energy = wpool.tile([128, H], bf16, tag="energy")
                        nc.scalar.activation(energy[:], p_proj[:], act.Tanh)
                        scr = wpool.tile([128, H], bf16, tag="scr")
                        if _USE_TTR:
                            nc.vector.tensor_tensor_reduce(
                                out=scr[:], in0=energy[:], in1=wv_bc[:],
                                scale=1.0, scalar=0.0,
                                op0=alu.mult, op1=alu.add,
                                accum_out=s_sb[:, stp:stp + 1],
                            )
                        else:
                            nc.vector.tensor_mul(scr[:], energy[:], wv_bc[:])
                            nc.vector.reduce_sum(
                                s_sb[:, stp:stp + 1], scr[:],
                                axis=mybir.AxisListType.X,
                            )
                        if stp == 3 and pend:
                            _emit_context(nc, mybir, ctx_pools, pend, ctx_d)
                            pend = []

                    if debug_dumps:
                        nc.sync.dma_start(
                            dbg_d[b].rearrange("(st p) -> p st", p=128), s_sb[:]
                        )
                    # ---- softmax over the 1024 s of batch b ----
                    sm = bpool.tile([128, ST], f32, tag="sm")
                    nc.vector.tensor_add(
                        sm[:], s_sb[:], mbias[:, b * ST:(b + 1) * ST]
                    )
                    p_exp = bpool.tile([128, ST], f32, tag="p_exp")
                    rowsum = bpool.tile([128, 1], f32, tag="rowsum")
                    nc.scalar.activation(p_exp[:], sm[:], act.Exp,
                                         accum_out=rowsum[:])
                    # bf16 copy of p_exp for the context matmul
                    p_r16 = bpool.tile([128, ST], bf16, tag="p_r16")
                    nc.vector.tensor_copy(p_r16[:], p_exp[:])
                    p_den = psmall_pool.tile([1, 1], f32, tag="small")
                    nc.tensor.matmul(p_den[:], rowsum[:], ones_col_f[:],
                                     start=True, stop=True)
                    rd = bpool.tile([1, 1], f32, tag="rd")
                    nc.vector.reciprocal(rd[:], p_den[:])
                    p_rb = psmall_pool.tile([128, 1], f32, tag="small")
                    nc.tensor.matmul(p_rb[:], ones_row_f[:], rd[:],
                                     start=True, stop=True)
                    rb = bpool.tile([128, 1], f32, tag="rb")
                    nc.any.tensor_copy(rb[:], p_rb[:])
                    attn_sb = bpool.tile([128, ST], f32, tag="attn_sb")
                    nc.vector.tensor_scalar_mul(attn_sb[:], p_exp[:], rb[:])
                    nc.sync.dma_start(
                        attn_d[b].rearrange("(st p) -> p st", p=128), attn_sb[:]
                    )
                    pend.append((b, enc_b, p_r16, rb))
                _emit_context(nc, mybir, ctx_pools, pend, ctx_d)
                pend = []

    _split_multiwaits(nc)
    return nc


def _get_nc(n_iters: int = 1):
    key = ("nc", n_iters)
    if key not in _cache:
        _cache[key] = build_kernel(n_iters)
    return _cache[key]


def shard_inputs(hidden, encoder_outputs, mask, W_attn, b_attn, W_v):
    hidden = np.ascontiguousarray(np.asarray(hidden, dtype=np.float32))
    enc = np.ascontiguousarray(np.asarray(encoder_outputs, dtype=np.float32))
    mask = np.ascontiguousarray(np.asarray(mask, dtype=np.int32))
    W_attn = np.ascontiguousarray(np.asarray(W_attn, dtype=np.float32))
    b_attn = np.ascontiguousarray(np.asarray(b_attn, dtype=np.float32))
    W_v = np.ascontiguousarray(np.asarray(W_v, dtype=np.float32))
    in_maps = []
    for c in range(N_CORES):
        sl = slice(c * B_LOC, (c + 1) * B_LOC)
        in_maps.append({
            "hidden": hidden[sl],
            "enc": enc[sl],
            "mask": mask[sl],
            "w_attn": W_attn,
            "b_attn": b_attn,
            "w_v": W_v,
        })
    return in_maps


def kernel(hidden, encoder_outputs, mask, W_attn, b_attn, W_v):
    from concourse.bass_utils import run_bass_kernel_spmd

    nc = _get_nc(1)
    in_maps = shard_inputs(hidden, encoder_outputs, mask, W_attn, b_attn, W_v)
    res = run_bass_kernel_spmd(nc, in_maps, list(range(N_CORES)))
    context = np.concatenate([res.results[c]["out_ctx"] for c in range(N_CORES)], 0)
    attn_w = np.concatenate([res.results[c]["out_attn"] for c in range(N_CORES)], 0)
    return context.astype(np.float32), attn_w.astype(np.float32)
